# revision 29
# baseline (speedup 1.0000x reference)
"""Multi-head causal attention with RoPE on 8 Trainium2 NeuronCores.

Sharding: data-parallel over batch (2 groups of 4 cores) x tensor-parallel
over heads (4 heads / 512 cols of Wq/Wk/Wv per core, 512 rows of Wo).
Each core computes its head-group's Q/K/V projections in transposed layout
([head_dim, seq] -- so no on-device transposes are ever needed), applies
RoPE, runs causal softmax attention (scores kept transposed [tk, tq];
row sums via a ones-vector matmul), and emits its partial output
projection.  The host sums the 4 partials per batch element.

Pipeline structure (per tq-chunk c): K/V projections for chunk c ->
Q projection for c -> attention rows c (consuming K/V tiles 0..c) ->
partial output projection rows c.  Startup DMAs are split per-d-tile and
interleaved so the first matmul can start ~1us in instead of waiting for
all constants.  Causal score blocks are column-sliced: only tq >= tk
columns are computed, with a single shared 128-wide triangular strip
pattern for the diagonal.

Self-contained: shapes/sharding hardcoded for
  q_input/kv_input [2, 2048, 2048], 16 heads x 128 head_dim.
"""

import math

import numpy as np
import ml_dtypes

B, T, D, H = 2, 2048, 2048, 16
HD = 128          # head dim
HALF = HD // 2    # rope half
P = 128           # partitions
CHUNK = 512       # tq / free-dim chunk
NCORES = 8
GROUPS = 4        # head-groups (tensor-parallel degree per batch)
HPG = H // GROUPS # heads per group
GD = HPG * HD     # group width (512)
DT = D // P       # d-tiles (16)
TCH = T // CHUNK  # seq chunks (4)
TKT = T // P      # tk tiles (16)
CPT = CHUNK // P  # tk tiles per chunk (4)

TRACE = False       # set True before calling kernel() to capture an NTFF trace
LAST_RESULT = None  # BassKernelResults of the last kernel() call

_cache = {}


def _build_program(actions, npat1, npat2, repeat=1):
    """Build the per-core Bass program.

    actions: {(c, t): ("plain", lo) | ("strip", lo, idx) | ("wide", idx)}
    for every (tq-chunk, tk-tile) score block with >=1 unmasked element:
      - plain: columns [lo, CHUNK) fully unmasked, columns < lo fully masked
      - strip: like plain but columns [lo, lo+128) need pat128[idx]
      - wide:  full-width pattern pat512[idx]
    repeat: unroll the whole body N times (for differential timing in bench.py).
    """
    from contextlib import ExitStack

    import concourse.mybir as mybir
    import concourse.tile as tile
    from concourse import bacc
    from concourse.bass import ts

    fp32 = mybir.dt.float32
    fp16 = mybir.dt.float16
    bf16 = mybir.dt.bfloat16
    Copy = mybir.ActivationFunctionType.Copy
    Exp = mybir.ActivationFunctionType.Exp
    SCALE = 1.0 / math.sqrt(HD)

    nc = bacc.Bacc(
        "TRN2",
        target_bir_lowering=False,
        debug=False,
        enable_asserts=False,
        num_devices=NCORES,
    )

    xqT = nc.dram_tensor("xqT", [D, T], bf16, kind="ExternalInput").ap()
    xkvT = nc.dram_tensor("xkvT", [D, T], bf16, kind="ExternalInput").ap()
    wq = nc.dram_tensor("wq", [D, GD], bf16, kind="ExternalInput").ap()
    wk = nc.dram_tensor("wk", [D, GD], bf16, kind="ExternalInput").ap()
    wv = nc.dram_tensor("wv", [D, GD], bf16, kind="ExternalInput").ap()
    wo = nc.dram_tensor("wo", [GD, D], bf16, kind="ExternalInput").ap()
    # RoPE in head-dim-interleaved space (host permutes Wq/Wk columns so the
    # rope pair (j, j+64) lands on adjacent partitions (2j, 2j+1); scores are
    # invariant to a common Q/K head-dim permutation):
    #   rope'(x) = x * cs2 + swap_adjacent_pairs(x) * ss2
    # cs2[2j] = cs2[2j+1] = cos_j ; ss2[2j] = -sin_j, ss2[2j+1] = +sin_j
    cs2 = nc.dram_tensor("cs2", [P, T], bf16, kind="ExternalInput").ap()
    ss2 = nc.dram_tensor("ss2", [P, T], bf16, kind="ExternalInput").ap()
    pat1 = nc.dram_tensor("pat1", [npat1, P, P], bf16, kind="ExternalInput").ap()
    pat2 = nc.dram_tensor("pat2", [npat2, P, CHUNK], bf16, kind="ExternalInput").ap()
    out = nc.dram_tensor("out", [T, D], fp16, kind="ExternalOutput").ap()

    xkvr = xkvT.rearrange("(dt p) t -> p dt t", p=P)
    xqr = xqT.rearrange("(dt p) t -> p dt t", p=P)
    wqr = wq.rearrange("(dt p) n -> p dt n", p=P)
    wkr = wk.rearrange("(dt p) n -> p dt n", p=P)
    wvr = wv.rearrange("(dt p) n -> p dt n", p=P)
    wor = wo.rearrange("(h p) n -> p h n", p=P)

    tlists = {}
    for c in range(TCH):
        tlists[c] = sorted(t for (cc, t) in actions if cc == c)

    with ExitStack() as ctx:
        tc = ctx.enter_context(tile.TileContext(nc))
        const_pool = ctx.enter_context(tc.tile_pool(name="const", bufs=1))
        xpool = ctx.enter_context(tc.tile_pool(name="xchunk", bufs=2))
        qa_pool = ctx.enter_context(tc.tile_pool(name="qa", bufs=2))
        rope_pool = ctx.enter_context(tc.tile_pool(name="rope", bufs=2))
        exp_pool = ctx.enter_context(tc.tile_pool(name="exp", bufs=3))
        osb_pool = ctx.enter_context(tc.tile_pool(name="osb", bufs=4))
        lb_pool = ctx.enter_context(tc.tile_pool(name="lb", bufs=2))
        mm_psum = ctx.enter_context(tc.tile_pool(name="mmps", bufs=2, space="PSUM"))
        s_psum = ctx.enter_context(tc.tile_pool(name="sps", bufs=3, space="PSUM"))
        o_psum = ctx.enter_context(tc.tile_pool(name="ops", bufs=2, space="PSUM"))
        l_psum = ctx.enter_context(tc.tile_pool(name="lps", bufs=1, space="PSUM"))

        # persistent SBUF tensors
        wq_sb = const_pool.tile([P, DT, GD], bf16, tag="wq")
        wk_sb = const_pool.tile([P, DT, GD], bf16, tag="wk")
        wv_sb = const_pool.tile([P, DT, GD], bf16, tag="wv")
        wo_sb = const_pool.tile([P, HPG, D], bf16, tag="wo")
        cs2_sb = const_pool.tile([P, T], bf16, tag="cs2")
        ss2_sb = const_pool.tile([P, T], bf16, tag="ss2")
        use_wide = any(a[0] == "wide" for a in actions.values())
        pat1_sb = const_pool.tile([P, npat1, P], bf16, tag="pat1")
        pat2_sb = (
            const_pool.tile([P, npat2, CHUNK], bf16, tag="pat2") if use_wide else None
        )
        ones_sb = const_pool.tile([P, 1], bf16, tag="ones")
        KT = const_pool.tile([P, HPG, T], bf16, tag="KT")
        V = const_pool.tile([P, TKT, GD], bf16, tag="V")

        nc.vector.memset(ones_sb[:], 1.0)

        SHUF_MASK = [i + 1 - 2 * (i % 2) for i in range(32)]  # [1,0,3,2,...]

        def rope_evict(ps, c, dest):
            # ps: PSUM [P, CHUNK] fp32, partitions = interleaved head_dim
            # dest = ps * cs2[chunk] + swap_adjacent_pairs(ps) * ss2[chunk]
            rsw = rope_pool.tile([P, CHUNK], fp32, tag="rsw")
            nc.vector.stream_shuffle(rsw[:], ps[:], SHUF_MASK)
            nc.vector.tensor_mul(dest, ps[:], cs2_sb[:, ts(c, CHUNK)])
            t2 = rope_pool.tile([P, CHUNK], bf16, tag="t2")
            nc.vector.tensor_mul(t2[:], rsw[:], ss2_sb[:, ts(c, CHUNK)])
            nc.vector.tensor_add(dest, dest, t2[:])

        for _rep in range(repeat):
            # ---- startup: interleave per-d-tile weight/x DMAs so the first
            # projection matmul only waits for one 128x512 slice of each.
            # second DMA ring (Activation HWDGE): constants that aren't on the
            # critical startup path, in need-by order
            nc.scalar.dma_start(cs2_sb[:], cs2)
            nc.scalar.dma_start(ss2_sb[:], ss2)
            nc.scalar.dma_start(wq_sb[:], wqr)
            nc.scalar.dma_start(pat1_sb[:], pat1.rearrange("j p n -> p j n"))
            if use_wide:
                nc.scalar.dma_start(pat2_sb[:], pat2.rearrange("j p n -> p j n"))
            nc.scalar.dma_start(wo_sb[:], wor)
            # primary ring: per-d-tile triples pacing the chunk-0 projections
            xk0 = xpool.tile([P, DT, CHUNK], bf16, tag="xk")
            for d in range(DT):
                nc.sync.dma_start(wk_sb[:, d, :], wkr[:, d, :])
                nc.sync.dma_start(xk0[:, d, :], xkvr[:, d, ts(0, CHUNK)])
                nc.sync.dma_start(wv_sb[:, d, :], wvr[:, d, :])
            xq0 = xpool.tile([P, DT, CHUNK], bf16, tag="xq")
            for d in range(DT):
                nc.sync.dma_start(xq0[:, d, :], xqr[:, d, ts(0, CHUNK)])

            xk_next, xq_next = xk0, xq0
            for c in range(TCH):
                # ---- K^T and V for chunk c
                xk = xk_next
                # during the DMA-paced chunk 0, spread accumulation groups
                # over the idle attention PSUM banks so more matmuls are
                # ready per arriving d-slice
                def kproj_ps(h):
                    if c == 0 and h >= 2:
                        ps = s_psum.tile([P, CHUNK], fp32, tag="s")
                    else:
                        ps = mm_psum.tile([P, CHUNK], fp32, tag="mm")
                    return ps

                def vproj_ps(s):
                    if c == 0 and s < 2:
                        ps = o_psum.tile([P, CHUNK], fp32, tag="o")
                    elif c == 0 and s == 2:
                        ps = s_psum.tile([P, CHUNK], fp32, tag="s")
                    else:
                        ps = mm_psum.tile([P, GD], fp32, tag="mm")
                    return ps

                for h in range(HPG):
                    ps = kproj_ps(h)
                    for d in range(DT):
                        nc.tensor.matmul(
                            ps[:], wk_sb[:, d, ts(h, HD)], xk[:, d, :],
                            start=(d == 0), stop=(d == DT - 1),
                        )
                    rope_evict(ps, c, KT[:, h, ts(c, CHUNK)])
                for s in range(CPT):
                    ps = vproj_ps(s)
                    for d in range(DT):
                        nc.tensor.matmul(
                            ps[:], xk[:, d, ts(s, P)], wv_sb[:, d, :],
                            start=(d == 0), stop=(d == DT - 1),
                        )
                    nc.scalar.activation(V[:, c * CPT + s, :], ps[:], Copy)

                # ---- Q^T for chunk c (per-chunk tile; consumed by attention c)
                xq = xq_next
                QT = qa_pool.tile([P, HPG, CHUNK], bf16, tag="QT")
                for h in range(HPG):
                    ps = kproj_ps(h)
                    for d in range(DT):
                        nc.tensor.matmul(
                            ps[:], wq_sb[:, d, ts(h, HD)], xq[:, d, :],
                            start=(d == 0), stop=(d == DT - 1),
                        )
                    rope_evict(ps, c, QT[:, h, :])

                # prefetch next chunk's inputs ahead of the out DMAs so the
                # next chunk's projections can fill the attention-tail bubble
                if c + 1 < TCH:
                    xk_next = xpool.tile([P, DT, CHUNK], bf16, tag="xk")
                    nc.sync.dma_start(xk_next[:], xkvr[:, :, ts(c + 1, CHUNK)])
                    xq_next = xpool.tile([P, DT, CHUNK], bf16, tag="xq")
                    nc.sync.dma_start(xq_next[:], xqr[:, :, ts(c + 1, CHUNK)])

                # ---- attention rows c (scores kept transposed [tk, tq];
                # causal col-slicing: block (c,t) only computes tq cols >= lo)
                AT = qa_pool.tile([P, HPG, CHUNK], bf16, tag="AT")
                # In all but the last chunk the attention pass has PE slack:
                # accumulate es_sum on DVE (fp16) and use ONE ones-matmul for
                # the row sums.  The last chunk is eviction(ACT)-bound, so its
                # per-t ones-matmuls ride in otherwise-idle PE slots for free.
                dve_lsum = True
                for h in range(HPG):
                    opst = o_psum.tile([P, CHUNK], fp32, tag="o")
                    lpst = l_psum.tile([1, CHUNK], fp32, tag="l")
                    if dve_lsum:
                        es_sum = lb_pool.tile([P, CHUNK], fp16, tag="es_sum")
                    tlist = tlists[c]
                    for i, t in enumerate(tlist):
                        act = actions[(c, t)]
                        if act[0] == "wide":
                            lo = 0
                        else:
                            lo = act[1]
                        n = CHUNK - lo
                        spst = s_psum.tile([P, CHUNK], fp32, tag="s")
                        nc.tensor.matmul(
                            spst[:, lo:], KT[:, h, ts(t, P)],
                            QT[:, h, lo:], start=True, stop=True,
                        )
                        es = exp_pool.tile([P, CHUNK], bf16, tag="es")
                        nc.scalar.activation(es[:, lo:], spst[:, lo:], Exp, scale=SCALE)
                        if act[0] == "strip":
                            w = min(P, n)
                            nc.gpsimd.tensor_mul(
                                es[:, lo:lo + w], es[:, lo:lo + w],
                                pat1_sb[:, act[2], :w],
                            )
                        elif act[0] == "wide":
                            nc.gpsimd.tensor_mul(es[:], es[:], pat2_sb[:, act[1], :])
                        first, last = (i == 0), (i == len(tlist) - 1)
                        assert not first or lo == 0
                        if dve_lsum:
                            if first:
                                nc.vector.tensor_copy(es_sum[:], es[:])
                            else:
                                nc.vector.tensor_add(
                                    es_sum[:, lo:], es_sum[:, lo:], es[:, lo:]
                                )
                        else:
                            nc.tensor.matmul(
                                lpst[:, lo:], ones_sb[:], es[:, lo:],
                                start=first, stop=last,
                            )
                        nc.tensor.matmul(
                            opst[:, lo:], V[:, t, ts(h, HD)], es[:, lo:],
                            start=first, stop=last,
                        )
                    if dve_lsum:
                        nc.tensor.matmul(
                            lpst[:], ones_sb[:], es_sum[:], start=True, stop=True
                        )
                    rec = lb_pool.tile([1, CHUNK], fp32, tag="rec")
                    nc.vector.reciprocal(rec[:], lpst[:])
                    # broadcast 1/l across partitions on GpSimd
                    lbs = lb_pool.tile([P, CHUNK], fp32, tag="lbs")
                    nc.gpsimd.partition_broadcast(lbs[:], rec[:])
                    nc.vector.tensor_mul(AT[:, h, :], opst[:], lbs[:])

                # ---- partial output projection for rows c:
                # out[tq, :] = sum_h attn_h^T.T @ Wo_h
                for m in range(CPT):
                    for oc in range(D // CHUNK):
                        # tag "s": share the attention score pool's banks so
                        # mm_psum stays free for the next chunk's projections
                        ps = s_psum.tile([P, CHUNK], fp32, tag="s")
                        for h in range(HPG):
                            nc.tensor.matmul(
                                ps[:], AT[:, h, ts(m, P)], wo_sb[:, h, ts(oc, CHUNK)],
                                start=(h == 0), stop=(h == HPG - 1),
                            )
                        ob = osb_pool.tile([P, CHUNK], fp16, tag="ob")
                        nc.vector.tensor_copy(ob[:], ps[:])
                        nc.sync.dma_start(
                            out[ts(c * CPT + m, P), ts(oc, CHUNK)], ob[:]
                        )

    nc.compile()
    return nc


def _interleave_heads(W):
    """Permute each 128-wide head block of columns: new[2j]=old[j], new[2j+1]=old[64+j]."""
    d, gd = W.shape
    return np.ascontiguousarray(
        W.reshape(d, gd // HD, 2, HALF).transpose(0, 1, 3, 2).reshape(d, gd)
    )


def _rope_tables(cos, sin):
    """cs2[2j]=cs2[2j+1]=cos_j ; ss2[2j]=-sin_j, ss2[2j+1]=+sin_j  (both [128, T])."""
    bf = ml_dtypes.bfloat16
    cosT = np.ascontiguousarray(cos.T)  # [HALF, T]
    sinT = np.ascontiguousarray(sin.T)
    cs2 = np.repeat(cosT, 2, axis=0).astype(bf)
    ss2 = np.stack([-sinT, sinT], axis=1).reshape(HD, -1).astype(bf)
    return cs2, ss2


def _mask_actions(mask):
    """Classify every [CHUNK tq x P tk] score block of the mask.

    Returns (actions, pat128 [npat1,P,P], pat512 [npat2,P,CHUNK]); see
    _build_program for the action encoding.  Patterns are stored transposed
    ([tk, tq]) to match the score layout.  Blocks with no unmasked element
    are omitted (skipped entirely).
    """
    bf = ml_dtypes.bfloat16
    m = np.asarray(mask).reshape(T, T).astype(bool)
    actions = {}
    pats1, pat1_keys = [], {}
    pats2, pat2_keys = [], {}

    def wide(bt):
        key = bt.tobytes()
        if key not in pat2_keys:
            pat2_keys[key] = len(pats2)
            pats2.append(bt.astype(bf))
        return ("wide", pat2_keys[key])

    for c in range(TCH):
        first_in_row = True
        for t in range(TKT):
            blk = m[c * CHUNK:(c + 1) * CHUNK, t * P:(t + 1) * P]
            if not blk.any():
                continue
            bt = np.ascontiguousarray(blk.T)  # [tk, tq]
            colact = bt.any(axis=0)
            lo = int(np.argmax(colact))
            if not colact[lo:].all() or (first_in_row and lo > 0):
                actions[(c, t)] = wide(bt)
            else:
                w = min(P, CHUNK - lo)
                strip = bt[:, lo:lo + w]
                rest = bt[:, lo + w:]
                if not rest.all():
                    actions[(c, t)] = wide(bt)
                elif strip.all():
                    actions[(c, t)] = ("plain", lo)
                else:
                    sp = np.ones((P, P), bf)
                    sp[:, :w] = strip.astype(bf)
                    key = sp.tobytes()
                    if key not in pat1_keys:
                        pat1_keys[key] = len(pats1)
                        pats1.append(sp)
                    actions[(c, t)] = ("strip", lo, pat1_keys[key])
            first_in_row = False
    if not pats1:
        pats1.append(np.zeros((P, P), bf))
    if not pats2:
        pats2.append(np.zeros((P, CHUNK), bf))
    return actions, np.ascontiguousarray(np.stack(pats1)), np.ascontiguousarray(np.stack(pats2))


def kernel(**inputs):
    global LAST_RESULT
    q_input = np.asarray(inputs["q_input"], dtype=np.float32)
    kv_input = np.asarray(inputs["kv_input"], dtype=np.float32)
    cos = np.asarray(inputs["cos"], dtype=np.float32)
    sin = np.asarray(inputs["sin"], dtype=np.float32)
    Wq = np.asarray(inputs["Wq"], dtype=np.float32)
    Wk = np.asarray(inputs["Wk"], dtype=np.float32)
    Wv = np.asarray(inputs["Wv"], dtype=np.float32)
    Wo = np.asarray(inputs["Wo"], dtype=np.float32)

    actions, pats1, pats2 = _mask_actions(inputs["mask"])
    key = (tuple(sorted(actions.items())), pats1.shape[0], pats2.shape[0])
    if key not in _cache:
        _cache[key] = _build_program(
            actions, int(pats1.shape[0]), int(pats2.shape[0])
        )
    nc = _cache[key]

    bf = ml_dtypes.bfloat16
    cs2, ss2 = _rope_tables(cos, sin)
    xq = [np.ascontiguousarray(q_input[b].T).astype(bf) for b in range(B)]
    xkv = [np.ascontiguousarray(kv_input[b].T).astype(bf) for b in range(B)]
    wq_g = [_interleave_heads(Wq[:, g * GD:(g + 1) * GD]).astype(bf) for g in range(GROUPS)]
    wk_g = [_interleave_heads(Wk[:, g * GD:(g + 1) * GD]).astype(bf) for g in range(GROUPS)]
    wv_g = [np.ascontiguousarray(Wv[:, g * GD:(g + 1) * GD]).astype(bf) for g in range(GROUPS)]
    wo_g = [np.ascontiguousarray(Wo[g * GD:(g + 1) * GD, :]).astype(bf) for g in range(GROUPS)]

    in_maps = []
    for core in range(NCORES):
        b, g = divmod(core, GROUPS)
        in_maps.append({
            "xqT": xq[b],
            "xkvT": xkv[b],
            "wq": wq_g[g],
            "wk": wk_g[g],
            "wv": wv_g[g],
            "wo": wo_g[g],
            "cs2": cs2,
            "ss2": ss2,
            "pat1": pats1,
            "pat2": pats2,
        })

    from concourse import bass_utils

    res = bass_utils.run_bass_kernel_spmd(
        nc, in_maps, core_ids=list(range(NCORES)), trace=TRACE
    )
    LAST_RESULT = res
    outs = [r["out"] for r in res.results]
    full = np.stack(
        [sum(outs[b * GROUPS + g] for g in range(GROUPS)) for b in range(B)]
    )
    return np.ascontiguousarray(full.astype(np.float32))
